# revision 24
# baseline (speedup 1.0000x reference)
"""AVWGCN graph-conv kernel for 8x Trainium2 NeuronCores (Bass/Tile).

Problem (B=32, N=4096, D=16, K=2, CIN=COUT=32):
  supports = softmax(relu(E @ E.T), axis=1)            # [N, N]
  W        = einsum('nd,dkio->nkio', E, Wp)            # per-node weights
  bias     = E @ bias_pool                             # [N, COUT]
  x_g      = stack([x, supports @ x], axis=k)          # [B, N, K, CIN]
  out      = einsum('bnki,nkio->bno', x_g, W) + bias

Sharding: row-parallel over N - each core owns NL=512 nodes (all batches),
x replicated (full m-contraction on every core). Host gathers by concat
along N. No collectives.

Per-core pipeline (layouts chosen so no on-device transposes are needed):
  A:  expA_T[m, n] = exp(relu(E[m].E_loc[n]))  PE f32r + ACT Exp + DVE max(.,1)
      software-pipelined with lag 2 so PE never waits on the Exp latency;
      Z[n] column-sums (ones-vector matmul) and the first two U bichunks
      ride in the same m-loop to keep PE dense (HAM stays warm).
  U:  U_T[(b,i), n] = sum_m X[m,(b,i)] expA_T[m,n]   bf16 PE, j-outer loop,
      double-buffered single-bank PSUM accumulators; evac multiplies 1/Z.
  Y:  Y[n, b, (d,o)] = sum_i xT[(b,i),n] Wp0[i,(d,o)] + U_T[(b,i),n] Wp1[i,(d,o)]
      4 batches concurrent on 32-row PE strips (tile_position); pairs of
      batches share a 2-bank PSUM tile so one wide ACT copy evacuates both.
  R:  out[n, b, o] = sum_d Y[n,b,(d,o)] E[n,d] + bias[n,o]
      DVE fp16 2x broadcast multiply + d-outer tree reduce + bias add.
"""

import os

import ml_dtypes
import numpy as np

import concourse.bass as bass
import concourse.tile as tile
from concourse import bacc, mybir
from concourse.bass_utils import run_bass_kernel_spmd

BF16 = ml_dtypes.bfloat16

B, N, D, CIN, COUT = 32, 4096, 16, 32, 32
NC = 8                  # cores
NL = N // NC            # nodes per core = 512
MC = N // 128           # m-chunks = 32
BI = B * CIN            # 1024
NJ = BI // 128          # bichunks = 8
NQ = NL // 128          # n-chunks per core = 4
DO = COUT * D           # 512, free layout (d, o) with o innermost

F32 = mybir.dt.float32
F32R = mybir.dt.float32r
BF16_DT = mybir.dt.bfloat16
FP16_DT = mybir.dt.float16

A_MM_DT = F32 if os.environ.get("AVW_A_F32", "0") == "1" else F32R
LAG = 3                 # phase-A software pipeline depth


def _build_nc():
    nc = bacc.Bacc("TRN2", target_bir_lowering=False, debug=False, num_devices=NC)

    d_xall = nc.dram_tensor("xall_bi", [N, BI], BF16_DT, kind="ExternalInput").ap()
    d_xlocT = nc.dram_tensor("xlocT", [BI, NL], BF16_DT, kind="ExternalInput").ap()
    d_efr = nc.dram_tensor("efr", [D, N], F32R, kind="ExternalInput").ap()
    d_elr = nc.dram_tensor("elr", [D, NL], F32R, kind="ExternalInput").ap()
    d_elocT32 = nc.dram_tensor("elocT32", [D, NL], F32, kind="ExternalInput").ap()
    d_erep = nc.dram_tensor("erep", [NL, DO], FP16_DT, kind="ExternalInput").ap()
    d_wp0 = nc.dram_tensor("wp0", [128, DO], BF16_DT, kind="ExternalInput").ap()
    d_wp1 = nc.dram_tensor("wp1", [128, DO], BF16_DT, kind="ExternalInput").ap()
    d_biasp = nc.dram_tensor("biaspool", [D, COUT], F32, kind="ExternalInput").ap()
    d_out = nc.dram_tensor("out_loc", [B, NL, COUT], F32, kind="ExternalOutput").ap()
    d_zscr = nc.dram_tensor("zscratch", [1, NL], F32).ap()

    with tile.TileContext(nc) as tc:
        with (
            tc.tile_pool(name="consts", bufs=1) as consts,
            tc.tile_pool(name="big", bufs=1) as big,
            tc.tile_pool(name="ystage", bufs=4) as ystage,
            tc.tile_pool(name="ostage", bufs=4) as ostage,
            tc.tile_pool(name="pu", bufs=3, space="PSUM") as pu_pool,
        ):
            # ---- resident SBUF tensors ----
            sb_efr = consts.tile([D, N], F32R, tag="efr")
            nc.sync.dma_start(out=sb_efr[:], in_=d_efr)
            sb_elr = consts.tile([D, NL], F32R, tag="elr")
            nc.sync.dma_start(out=sb_elr[:], in_=d_elr)
            sb_elocT32 = consts.tile([D, NL], F32, tag="elocT32")
            nc.sync.dma_start(out=sb_elocT32[:], in_=d_elocT32)
            sb_biasp = consts.tile([D, COUT], F32, tag="biasp")
            nc.sync.dma_start(out=sb_biasp[:], in_=d_biasp)
            sb_wp0 = consts.tile([128, DO], BF16_DT, tag="wp0")
            nc.sync.dma_start(out=sb_wp0[:], in_=d_wp0)
            sb_wp1 = consts.tile([128, DO], BF16_DT, tag="wp1")
            nc.sync.dma_start(out=sb_wp1[:], in_=d_wp1)
            sb_erep = consts.tile([128, NQ, DO], FP16_DT, tag="erep")
            nc.sync.dma_start(
                out=sb_erep[:], in_=d_erep.rearrange("(q p) od -> p q od", p=128)
            )
            sb_xlocT = consts.tile([128, NJ, NL], BF16_DT, tag="xlocT")
            nc.sync.dma_start(
                out=sb_xlocT[:], in_=d_xlocT.rearrange("(j p) n -> p j n", p=128)
            )
            sb_ones = consts.tile([128, 1], BF16_DT, tag="ones")
            nc.vector.memset(sb_ones[:], 1.0)
            sb_bias = consts.tile([128, NQ, COUT], F32, tag="bias")
            sb_zrep = consts.tile([128, NL], F32, tag="zrep")

            sb_xall = big.tile([128, MC, BI], BF16_DT, tag="xall")
            xall_r = d_xall.rearrange("(mc p) bi -> p mc bi", p=128)
            for mc in range(MC):
                nc.sync.dma_start(out=sb_xall[:, mc, :], in_=xall_r[:, mc, :])

            sb_expA = big.tile([128, MC, NL], BF16_DT, tag="expA")
            sb_u = big.tile([128, NJ, NL], BF16_DT, tag="u")

            # ---- phase A: pipelined A/exp/max + Z + U bichunks 0,1 ----
            with (
                tc.tile_pool(name="pa", bufs=3, space="PSUM") as pa_pool,
                tc.tile_pool(name="psm", bufs=1, space="PSUM") as psm_pool,
            ):
                # dense bf16 warmup burst: trips the HAM clock gate to
                # K=8/8 while input DMAs stream; result never read
                sb_warm = ystage.tile([128, 512], BF16_DT, tag="warm", name="sb_warm")
                nc.vector.memset(sb_warm[:], 0.0)
                pwarm = psm_pool.tile([128, 512], F32, tag="psm", name="pwarm")
                for w in range(14):
                    nc.tensor.matmul(
                        pwarm[:],
                        sb_warm[:, 0:128],
                        sb_warm[:],
                        start=True,
                        stop=True,
                    )

                for q in range(NQ):
                    pb = psm_pool.tile([128, COUT], F32, tag="psm", name=f"pb{q}")
                    nc.tensor.matmul(
                        pb[:],
                        sb_elocT32[:, q * 128 : (q + 1) * 128],
                        sb_biasp[:],
                        start=True,
                        stop=True,
                    )
                    nc.scalar.copy(out=sb_bias[:, q, :], in_=pb[:])

                pz = psm_pool.tile([1, NL], F32, tag="psm", name="pz")
                pu0 = pu_pool.tile([128, NL], F32, tag="pu", name="pu_0")
                pu1 = pu_pool.tile([128, NL], F32, tag="pu", name="pu_1")

                def a_stage(mc):
                    pa = pa_pool.tile([128, NL], F32, tag="pa", name=f"pa{mc}")
                    nc.tensor.matmul(
                        pa[:],
                        sb_efr[:, mc * 128 : (mc + 1) * 128],
                        sb_elr[:],
                        start=True,
                        stop=True,
                    )
                    nc.scalar.activation(
                        out=sb_expA[:, mc, :],
                        in_=pa[:],
                        func=mybir.ActivationFunctionType.Exp,
                    )
                    nc.gpsimd.tensor_scalar_max(
                        out=sb_expA[:, mc, :], in0=sb_expA[:, mc, :], scalar1=1.0
                    )

                def zu_stage(mc):
                    nc.tensor.matmul(
                        pz[:],
                        sb_ones[:],
                        sb_expA[:, mc, :],
                        start=(mc == 0),
                        stop=(mc == MC - 1),
                    )
                    for j in (0, 1):
                        nc.tensor.matmul(
                            (pu0, pu1)[j][:],
                            sb_xall[:, mc, j * 128 : (j + 1) * 128],
                            sb_expA[:, mc, :],
                            start=(mc == 0),
                            stop=(mc == MC - 1),
                        )

                for mc in range(MC + LAG):
                    if mc < MC:
                        a_stage(mc)
                    if mc >= LAG:
                        zu_stage(mc - LAG)

                # 1/Z on all partitions: DRAM-bounce broadcast, then ACT
                # reciprocal (table op, ~0.6us vs ~4us DVE reciprocal)
                sb_z1 = ystage.tile([1, NL], F32, tag="z1", name="sb_z1")
                nc.vector.tensor_copy(out=sb_z1[:], in_=pz[:])
                nc.sync.dma_start(out=d_zscr, in_=sb_z1[:])
                zbc = bass.AP(tensor=d_zscr.tensor, offset=0, ap=[[0, 128], [1, NL]])
                nc.sync.dma_start(out=sb_zrep[:], in_=zbc)
                nc.vector.reciprocal(out=sb_zrep[:], in_=sb_zrep[:])

                nc.vector.tensor_mul(sb_u[:, 0, :], pu0[:], sb_zrep[:])
                nc.vector.tensor_mul(sb_u[:, 1, :], pu1[:], sb_zrep[:])

            # ---- U bichunks 2..7 with Y units spread between blocks ----
            py_cm = tc.tile_pool(name="py", bufs=2, space="PSUM")
            py_pool = py_cm.__enter__()

            def y_unit(q, jp):
                """Final contraction for n-chunk q, batches b = 8*jp + boff,
                boff = 4*jj + sp + 2*h -> yh slot = 4*sp + 2*jj + h (pairs of
                strips share a 2-bank PSUM tile; slots of a pair adjacent)."""
                yh = ystage.tile([128, 8, DO], FP16_DT, tag="yh", name=f"yh{q}_{jp}")
                nsl = slice(q * 128, (q + 1) * 128)
                for jj in range(2):
                    j = 2 * jp + jj
                    for sp in range(2):
                        py = py_pool.tile(
                            [128, 2 * DO], F32, tag="py", name=f"py{q}_{j}_{sp}"
                        )
                        for h in range(2):
                            s = sp + 2 * h
                            lo, hi = 32 * s, 32 * (s + 1)
                            osl = slice(h * DO, (h + 1) * DO)
                            nc.tensor.matmul(
                                py[:, osl],
                                sb_xlocT[lo:hi, j, nsl],
                                sb_wp0[lo:hi, :],
                                start=True,
                                stop=False,
                                tile_position=(lo, 0),
                            )
                            nc.tensor.matmul(
                                py[:, osl],
                                sb_u[lo:hi, j, nsl],
                                sb_wp1[lo:hi, :],
                                start=False,
                                stop=True,
                                tile_position=(lo, 0),
                            )
                        slot = 4 * sp + 2 * jj
                        nc.scalar.copy(out=yh[:, slot : slot + 2, :], in_=py[:])
                # multiply by E[n, d] (broadcast over slots and o)
                ebase = sb_erep[:, q, :]
                ebc = bass.AP(
                    tensor=ebase.tensor,
                    offset=ebase.offset,
                    ap=[ebase.ap[0], [0, 8], [1, DO]],
                )
                nc.vector.tensor_mul(yh[:], yh[:], ebc)
                # tree-reduce over d (outer of (d, o): contiguous halves)
                y4 = yh[:].rearrange("p b (d o) -> p b d o", o=COUT)
                for half in (8, 4, 2, 1):
                    nc.vector.tensor_add(
                        y4[:, :, 0:half, :],
                        y4[:, :, 0:half, :],
                        y4[:, :, half : 2 * half, :],
                    )
                oh = ostage.tile([128, 8, COUT], F32, tag="oh", name=f"oh{q}_{jp}")
                bbase = sb_bias[:, q, :]
                bbc = bass.AP(
                    tensor=bbase.tensor,
                    offset=bbase.offset,
                    ap=[bbase.ap[0], [0, 8], [1, COUT]],
                )
                nc.vector.tensor_add(oh[:], y4[:, :, 0, :], bbc)
                # slot -> batch: b = 8*jp + 4*jj + sp + 2*h; oh slot-major
                # order (sp, jj, h, o) maps to dst strides (+1b, +4b, +2b)
                dst = d_out.rearrange("b (q p) o -> q p b o", p=128)[q]
                dstu = dst[:, 8 * jp : 8 * (jp + 1), :]
                bstep = dstu.ap[1][0]  # element stride between batches
                for sp in range(2):
                    dap = bass.AP(
                        tensor=dstu.tensor,
                        offset=dstu.offset + sp * bstep,
                        ap=[dstu.ap[0], [2 * bstep, 4], [1, 32]],
                    )
                    nc.sync.dma_start(out=dap, in_=oh[:, 4 * sp : 4 * sp + 4, :])

            # units become ready as their bichunk pair completes:
            # jp0 after phase A (j=0,1); jp after j=2jp+1
            schedule = {2: [(0, 0), (1, 0)], 3: [(2, 0), (3, 0)],
                        4: [(0, 1), (1, 1)], 5: [(2, 1), (3, 1)],
                        6: [(0, 2), (1, 2)], 7: [(2, 2), (3, 2)]}
            for j in range(2, NJ):
                pu = pu_pool.tile([128, NL], F32, tag="pu", name=f"pu_{j}")
                for mc in range(MC):
                    nc.tensor.matmul(
                        pu[:],
                        sb_xall[:, mc, j * 128 : (j + 1) * 128],
                        sb_expA[:, mc, :],
                        start=(mc == 0),
                        stop=(mc == MC - 1),
                    )
                nc.vector.tensor_mul(sb_u[:, j, :], pu[:], sb_zrep[:])
                for q, jp in schedule[j]:
                    y_unit(q, jp)
            for q in range(NQ):
                y_unit(q, 3)

            py_cm.__exit__(None, None, None)

    nc.compile()
    return nc


_CACHED = {}


def _get_nc():
    if "nc" not in _CACHED:
        _CACHED["nc"] = _build_nc()
    return _CACHED["nc"]


def _prep_inputs(x, weights_pool, bias_pool, node_embeddings):
    x = np.asarray(x, dtype=np.float32)
    wp = np.asarray(weights_pool, dtype=np.float32)
    bp = np.asarray(bias_pool, dtype=np.float32)
    E = np.asarray(node_embeddings, dtype=np.float32)

    xall = np.ascontiguousarray(x.transpose(1, 0, 2)).reshape(N, BI).astype(BF16)
    ET = np.ascontiguousarray(E.T)
    # wp_k[i, d*COUT+o] = Wp[d, k, i, o]  (d-outer, o-inner free layout)
    wp0 = np.tile(
        np.ascontiguousarray(wp[:, 0].transpose(1, 0, 2)).reshape(CIN, DO), (4, 1)
    ).astype(BF16)
    wp1 = np.tile(
        np.ascontiguousarray(wp[:, 1].transpose(1, 0, 2)).reshape(CIN, DO), (4, 1)
    ).astype(BF16)

    in_maps = []
    for c in range(NC):
        loc = slice(c * NL, (c + 1) * NL)
        elocT = np.ascontiguousarray(E[loc].T)
        in_maps.append(
            {
                "xall_bi": xall,
                "xlocT": np.ascontiguousarray(x[:, loc, :].transpose(0, 2, 1))
                .reshape(BI, NL)
                .astype(BF16),
                "efr": ET,
                "elr": elocT,
                "elocT32": elocT,
                "erep": np.repeat(E[loc], COUT, axis=1).astype(np.float16),
                "wp0": wp0,
                "wp1": wp1,
                "biaspool": bp,
            }
        )
    return in_maps


def _run(trace=False, **inputs):
    nc = _get_nc()
    in_maps = _prep_inputs(**inputs)
    res = run_bass_kernel_spmd(nc, in_maps, core_ids=list(range(NC)), trace=trace)
    out = np.concatenate([r["out_loc"] for r in res.results], axis=1)
    return out.astype(np.float32), res


def kernel(**inputs):
    out, _ = _run(trace=False, **inputs)
    return out


def run_traced(**inputs):
    out, res = _run(trace=True, **inputs)
    return out, res


# revision 25
# speedup vs baseline: 1.8664x; 1.8664x over previous
"""AVWGCN graph-conv kernel for 8x Trainium2 NeuronCores (Bass/Tile).

Problem (B=32, N=4096, D=16, K=2, CIN=COUT=32):
  supports = softmax(relu(E @ E.T), axis=1)            # [N, N]
  W        = einsum('nd,dkio->nkio', E, Wp)            # per-node weights
  bias     = E @ bias_pool                             # [N, COUT]
  x_g      = stack([x, supports @ x], axis=k)          # [B, N, K, CIN]
  out      = einsum('bnki,nkio->bno', x_g, W) + bias

Sharding: row-parallel over N - each core owns NL=512 nodes (all batches),
x replicated (full m-contraction on every core). Host gathers by concat
along N. No collectives.

Per-core pipeline (layouts chosen so no on-device transposes are needed):
  A:  expA_T[m, n] = exp(relu(E[m].E_loc[n]))  PE f32r + ACT Exp + DVE max(.,1)
      software-pipelined with lag 2 so PE never waits on the Exp latency;
      Z[n] column-sums (ones-vector matmul) and the first two U bichunks
      ride in the same m-loop to keep PE dense (HAM stays warm).
  U:  U_T[(b,i), n] = sum_m X[m,(b,i)] expA_T[m,n]   bf16 PE, j-outer loop,
      double-buffered single-bank PSUM accumulators; evac multiplies 1/Z.
  Y:  Y[n, b, (d,o)] = sum_i xT[(b,i),n] Wp0[i,(d,o)] + U_T[(b,i),n] Wp1[i,(d,o)]
      4 batches concurrent on 32-row PE strips (tile_position); pairs of
      batches share a 2-bank PSUM tile so one wide ACT copy evacuates both.
  R:  out[n, b, o] = sum_d Y[n,b,(d,o)] E[n,d] + bias[n,o]
      DVE fp16 2x broadcast multiply + d-outer tree reduce + bias add.
"""

import os

import ml_dtypes
import numpy as np

import concourse.bass as bass
import concourse.tile as tile
from concourse import bacc, mybir
from concourse.bass_utils import run_bass_kernel_spmd

BF16 = ml_dtypes.bfloat16

B, N, D, CIN, COUT = 32, 4096, 16, 32, 32
NC = 8                  # cores
NL = N // NC            # nodes per core = 512
MC = N // 128           # m-chunks = 32
BI = B * CIN            # 1024
NJ = BI // 128          # bichunks = 8
NQ = NL // 128          # n-chunks per core = 4
DO = COUT * D           # 512, free layout (d, o) with o innermost

F32 = mybir.dt.float32
F32R = mybir.dt.float32r
BF16_DT = mybir.dt.bfloat16
FP16_DT = mybir.dt.float16

A_MM_DT = F32 if os.environ.get("AVW_A_F32", "0") == "1" else F32R
LAG = 3                 # phase-A software pipeline depth


def _build_nc():
    nc = bacc.Bacc("TRN2", target_bir_lowering=False, debug=False, num_devices=NC)

    d_xall = nc.dram_tensor("xall_bi", [N, BI], BF16_DT, kind="ExternalInput").ap()
    d_xlocT = nc.dram_tensor("xlocT", [BI, NL], BF16_DT, kind="ExternalInput").ap()
    d_efr = nc.dram_tensor("efr", [D, N], F32R, kind="ExternalInput").ap()
    d_elr = nc.dram_tensor("elr", [D, NL], F32R, kind="ExternalInput").ap()
    d_elocT32 = nc.dram_tensor("elocT32", [D, NL], F32, kind="ExternalInput").ap()
    d_erep = nc.dram_tensor("erep", [NL, DO], FP16_DT, kind="ExternalInput").ap()
    d_wp0 = nc.dram_tensor("wp0", [128, DO], BF16_DT, kind="ExternalInput").ap()
    d_wp1 = nc.dram_tensor("wp1", [128, DO], BF16_DT, kind="ExternalInput").ap()
    d_biasp = nc.dram_tensor("biaspool", [D, COUT], F32, kind="ExternalInput").ap()
    d_out = nc.dram_tensor("out_loc", [B, NL, COUT], F32, kind="ExternalOutput").ap()
    d_zscr = nc.dram_tensor("zscratch", [1, NL], F32).ap()

    with tile.TileContext(nc) as tc:
        with (
            tc.tile_pool(name="consts", bufs=1) as consts,
            tc.tile_pool(name="big", bufs=1) as big,
            tc.tile_pool(name="ystage", bufs=4) as ystage,
            tc.tile_pool(name="ostage", bufs=4) as ostage,
            tc.tile_pool(name="pu", bufs=3, space="PSUM") as pu_pool,
        ):
            # ---- resident SBUF tensors ----
            sb_efr = consts.tile([D, N], F32R, tag="efr")
            nc.sync.dma_start(out=sb_efr[:], in_=d_efr)
            sb_elr = consts.tile([D, NL], F32R, tag="elr")
            nc.sync.dma_start(out=sb_elr[:], in_=d_elr)
            sb_elocT32 = consts.tile([D, NL], F32, tag="elocT32")
            nc.sync.dma_start(out=sb_elocT32[:], in_=d_elocT32)
            sb_biasp = consts.tile([D, COUT], F32, tag="biasp")
            nc.sync.dma_start(out=sb_biasp[:], in_=d_biasp)
            sb_wp0 = consts.tile([128, DO], BF16_DT, tag="wp0")
            nc.sync.dma_start(out=sb_wp0[:], in_=d_wp0)
            sb_wp1 = consts.tile([128, DO], BF16_DT, tag="wp1")
            nc.sync.dma_start(out=sb_wp1[:], in_=d_wp1)
            sb_erep = consts.tile([128, NQ, DO], FP16_DT, tag="erep")
            nc.sync.dma_start(
                out=sb_erep[:], in_=d_erep.rearrange("(q p) od -> p q od", p=128)
            )
            sb_xlocT = consts.tile([128, NJ, NL], BF16_DT, tag="xlocT")
            nc.sync.dma_start(
                out=sb_xlocT[:], in_=d_xlocT.rearrange("(j p) n -> p j n", p=128)
            )
            sb_ones = consts.tile([128, 1], BF16_DT, tag="ones")
            nc.vector.memset(sb_ones[:], 1.0)
            sb_bias = consts.tile([128, NQ, COUT], F32, tag="bias")
            sb_zrep = consts.tile([128, NL], F32, tag="zrep")

            sb_xall = big.tile([128, MC, BI], BF16_DT, tag="xall")
            xall_r = d_xall.rearrange("(mc p) bi -> p mc bi", p=128)
            for mc in range(MC):
                nc.sync.dma_start(out=sb_xall[:, mc, :], in_=xall_r[:, mc, :])

            sb_expA = big.tile([128, MC, NL], BF16_DT, tag="expA")
            sb_u = big.tile([128, NJ, NL], BF16_DT, tag="u")

            # ---- phase A: pipelined A/exp/max + Z + U bichunks 0,1 ----
            with (
                tc.tile_pool(name="pa", bufs=3, space="PSUM") as pa_pool,
                tc.tile_pool(name="psm", bufs=1, space="PSUM") as psm_pool,
            ):
                # dense bf16 warmup burst: trips the HAM clock gate to
                # K=8/8 while input DMAs stream; result never read
                sb_warm = ystage.tile([128, 512], BF16_DT, tag="warm", name="sb_warm")
                nc.vector.memset(sb_warm[:], 0.0)
                pwarm = psm_pool.tile([128, 512], F32, tag="psm", name="pwarm")
                for w in range(14):
                    nc.tensor.matmul(
                        pwarm[:],
                        sb_warm[:, 0:128],
                        sb_warm[:],
                        start=True,
                        stop=True,
                    )

                for q in range(NQ):
                    pb = psm_pool.tile([128, COUT], F32, tag="psm", name=f"pb{q}")
                    nc.tensor.matmul(
                        pb[:],
                        sb_elocT32[:, q * 128 : (q + 1) * 128],
                        sb_biasp[:],
                        start=True,
                        stop=True,
                    )
                    nc.scalar.copy(out=sb_bias[:, q, :], in_=pb[:])

                pz = psm_pool.tile([1, NL], F32, tag="psm", name="pz")
                pu0 = pu_pool.tile([128, NL], F32, tag="pu", name="pu_0")
                pu1 = pu_pool.tile([128, NL], F32, tag="pu", name="pu_1")

                def a_stage(mc):
                    pa = pa_pool.tile([128, NL], F32, tag="pa", name=f"pa{mc}")
                    nc.tensor.matmul(
                        pa[:],
                        sb_efr[:, mc * 128 : (mc + 1) * 128],
                        sb_elr[:],
                        start=True,
                        stop=True,
                    )
                    nc.scalar.activation(
                        out=sb_expA[:, mc, :],
                        in_=pa[:],
                        func=mybir.ActivationFunctionType.Exp,
                    )
                    nc.vector.tensor_scalar_max(
                        out=sb_expA[:, mc, :], in0=sb_expA[:, mc, :], scalar1=1.0
                    )

                def zu_stage(mc):
                    nc.tensor.matmul(
                        pz[:],
                        sb_ones[:],
                        sb_expA[:, mc, :],
                        start=(mc == 0),
                        stop=(mc == MC - 1),
                    )
                    for j in (0, 1):
                        nc.tensor.matmul(
                            (pu0, pu1)[j][:],
                            sb_xall[:, mc, j * 128 : (j + 1) * 128],
                            sb_expA[:, mc, :],
                            start=(mc == 0),
                            stop=(mc == MC - 1),
                        )

                for mc in range(MC + LAG):
                    if mc < MC:
                        a_stage(mc)
                    if mc >= LAG:
                        zu_stage(mc - LAG)

                # 1/Z on all partitions: DRAM-bounce broadcast, then ACT
                # reciprocal (table op, ~0.6us vs ~4us DVE reciprocal)
                sb_z1 = ystage.tile([1, NL], F32, tag="z1", name="sb_z1")
                nc.vector.tensor_copy(out=sb_z1[:], in_=pz[:])
                nc.sync.dma_start(out=d_zscr, in_=sb_z1[:])
                zbc = bass.AP(tensor=d_zscr.tensor, offset=0, ap=[[0, 128], [1, NL]])
                nc.sync.dma_start(out=sb_zrep[:], in_=zbc)
                nc.vector.reciprocal(out=sb_zrep[:], in_=sb_zrep[:])

                nc.vector.tensor_mul(sb_u[:, 0, :], pu0[:], sb_zrep[:])
                nc.vector.tensor_mul(sb_u[:, 1, :], pu1[:], sb_zrep[:])

            # ---- U bichunks 2..7 with Y units spread between blocks ----
            py_cm = tc.tile_pool(name="py", bufs=2, space="PSUM")
            py_pool = py_cm.__enter__()

            def y_unit(q, jp):
                """Final contraction for n-chunk q, batches b = 8*jp + boff,
                boff = 4*jj + sp + 2*h -> yh slot = 4*sp + 2*jj + h (pairs of
                strips share a 2-bank PSUM tile; slots of a pair adjacent)."""
                yh = ystage.tile([128, 8, DO], FP16_DT, tag="yh", name=f"yh{q}_{jp}")
                nsl = slice(q * 128, (q + 1) * 128)
                for jj in range(2):
                    j = 2 * jp + jj
                    for sp in range(2):
                        py = py_pool.tile(
                            [128, 2 * DO], F32, tag="py", name=f"py{q}_{j}_{sp}"
                        )
                        for h in range(2):
                            s = sp + 2 * h
                            lo, hi = 32 * s, 32 * (s + 1)
                            osl = slice(h * DO, (h + 1) * DO)
                            nc.tensor.matmul(
                                py[:, osl],
                                sb_xlocT[lo:hi, j, nsl],
                                sb_wp0[lo:hi, :],
                                start=True,
                                stop=False,
                                tile_position=(lo, 0),
                            )
                            nc.tensor.matmul(
                                py[:, osl],
                                sb_u[lo:hi, j, nsl],
                                sb_wp1[lo:hi, :],
                                start=False,
                                stop=True,
                                tile_position=(lo, 0),
                            )
                        slot = 4 * sp + 2 * jj
                        nc.scalar.copy(out=yh[:, slot : slot + 2, :], in_=py[:])
                # multiply by E[n, d] (broadcast over slots and o)
                ebase = sb_erep[:, q, :]
                ebc = bass.AP(
                    tensor=ebase.tensor,
                    offset=ebase.offset,
                    ap=[ebase.ap[0], [0, 8], [1, DO]],
                )
                nc.vector.tensor_mul(yh[:], yh[:], ebc)
                # tree-reduce over d (outer of (d, o): contiguous halves)
                y4 = yh[:].rearrange("p b (d o) -> p b d o", o=COUT)
                for half in (8, 4, 2, 1):
                    nc.vector.tensor_add(
                        y4[:, :, 0:half, :],
                        y4[:, :, 0:half, :],
                        y4[:, :, half : 2 * half, :],
                    )
                oh = ostage.tile([128, 8, COUT], F32, tag="oh", name=f"oh{q}_{jp}")
                bbase = sb_bias[:, q, :]
                bbc = bass.AP(
                    tensor=bbase.tensor,
                    offset=bbase.offset,
                    ap=[bbase.ap[0], [0, 8], [1, COUT]],
                )
                nc.vector.tensor_add(oh[:], y4[:, :, 0, :], bbc)
                # slot -> batch: b = 8*jp + 4*jj + sp + 2*h; oh slot-major
                # order (sp, jj, h, o) maps to dst strides (+1b, +4b, +2b)
                dst = d_out.rearrange("b (q p) o -> q p b o", p=128)[q]
                dstu = dst[:, 8 * jp : 8 * (jp + 1), :]
                bstep = dstu.ap[1][0]  # element stride between batches
                for sp in range(2):
                    dap = bass.AP(
                        tensor=dstu.tensor,
                        offset=dstu.offset + sp * bstep,
                        ap=[dstu.ap[0], [2 * bstep, 4], [1, 32]],
                    )
                    nc.sync.dma_start(out=dap, in_=oh[:, 4 * sp : 4 * sp + 4, :])

            # units become ready as their bichunk pair completes:
            # jp0 after phase A (j=0,1); jp after j=2jp+1
            schedule = {2: [(0, 0), (1, 0)], 3: [(2, 0), (3, 0)],
                        4: [(0, 1), (1, 1)], 5: [(2, 1), (3, 1)],
                        6: [(0, 2), (1, 2)], 7: [(2, 2), (3, 2)]}
            for j in range(2, NJ):
                pu = pu_pool.tile([128, NL], F32, tag="pu", name=f"pu_{j}")
                for mc in range(MC):
                    nc.tensor.matmul(
                        pu[:],
                        sb_xall[:, mc, j * 128 : (j + 1) * 128],
                        sb_expA[:, mc, :],
                        start=(mc == 0),
                        stop=(mc == MC - 1),
                    )
                nc.vector.tensor_mul(sb_u[:, j, :], pu[:], sb_zrep[:])
                for q, jp in schedule[j]:
                    y_unit(q, jp)
            for q in range(NQ):
                y_unit(q, 3)

            py_cm.__exit__(None, None, None)

    nc.compile()
    return nc


_CACHED = {}


def _get_nc():
    if "nc" not in _CACHED:
        _CACHED["nc"] = _build_nc()
    return _CACHED["nc"]


def _prep_inputs(x, weights_pool, bias_pool, node_embeddings):
    x = np.asarray(x, dtype=np.float32)
    wp = np.asarray(weights_pool, dtype=np.float32)
    bp = np.asarray(bias_pool, dtype=np.float32)
    E = np.asarray(node_embeddings, dtype=np.float32)

    xall = np.ascontiguousarray(x.transpose(1, 0, 2)).reshape(N, BI).astype(BF16)
    ET = np.ascontiguousarray(E.T)
    # wp_k[i, d*COUT+o] = Wp[d, k, i, o]  (d-outer, o-inner free layout)
    wp0 = np.tile(
        np.ascontiguousarray(wp[:, 0].transpose(1, 0, 2)).reshape(CIN, DO), (4, 1)
    ).astype(BF16)
    wp1 = np.tile(
        np.ascontiguousarray(wp[:, 1].transpose(1, 0, 2)).reshape(CIN, DO), (4, 1)
    ).astype(BF16)

    in_maps = []
    for c in range(NC):
        loc = slice(c * NL, (c + 1) * NL)
        elocT = np.ascontiguousarray(E[loc].T)
        in_maps.append(
            {
                "xall_bi": xall,
                "xlocT": np.ascontiguousarray(x[:, loc, :].transpose(0, 2, 1))
                .reshape(BI, NL)
                .astype(BF16),
                "efr": ET,
                "elr": elocT,
                "elocT32": elocT,
                "erep": np.repeat(E[loc], COUT, axis=1).astype(np.float16),
                "wp0": wp0,
                "wp1": wp1,
                "biaspool": bp,
            }
        )
    return in_maps


def _run(trace=False, **inputs):
    nc = _get_nc()
    in_maps = _prep_inputs(**inputs)
    res = run_bass_kernel_spmd(nc, in_maps, core_ids=list(range(NC)), trace=trace)
    out = np.concatenate([r["out_loc"] for r in res.results], axis=1)
    return out.astype(np.float32), res


def kernel(**inputs):
    out, _ = _run(trace=False, **inputs)
    return out


def run_traced(**inputs):
    out, res = _run(trace=True, **inputs)
    return out, res
